# revision 1
# baseline (speedup 1.0000x reference)
"""Batched matrix-attention scores kernel for Trainium2 (8 NeuronCores).

Computes scores[b, i, j] = sum_d m1[b, i, d] * m2[b, j, d]
  (i.e. jnp.einsum('bid,bjd->bij', matrix_1, matrix_2))
with B=16, R1=R2=2048, D=256, fp32 in/out.

Sharding: data-parallel over batch — 2 batches per core on 8 cores.

Per-core structure:
  - Operands are PE-transposed (matmul with identity) into
    D-on-partitions layout mT[d, dc, row] since the tensor engine
    contracts over the partition dim; fp32 can't use DMA transpose.
  - Each 128-row output tile takes 8 matmuls (2 d-chunks x 4 j-chunks
    of N=512, one fp32 PSUM bank each); PSUM is evacuated on DVE+ACT,
    output stored in 2MB blocks on the Sync DMA ring.
  - Batch b+1's loads (Scalar DMA ring) and transposes are interleaved
    into batch b's matmul phase so the store pipe never drains.

Operands use dt.float32r (fp32 bits, full-rate single-pass PE matmul;
~2^-11 input mantissa truncation) — ~4x the fp32 matmul rate.
Accumulation stays fp32 in PSUM.
"""

from contextlib import ExitStack

import numpy as np

import concourse.bass as bass
import concourse.mybir as mybir
import concourse.tile as tile
from concourse import bacc
from concourse.bass_utils import run_bass_kernel_spmd

F32 = mybir.dt.float32
F32R = mybir.dt.float32r

NCORES = 8
B, R1, R2, D = 16, 2048, 2048, 256
BPC = B // NCORES  # batches per core
P = 128
NJ_TILE = 512  # matmul free dim (one fp32 PSUM bank)
NJ = R2 // NJ_TILE  # j-chunks per row-block
NT = R1 // P  # 128-row tiles per batch
DC = D // P  # contraction chunks


def _build_tile_kernel(ctx: ExitStack, tc: tile.TileContext, m1, m2, ident_in, out):
    nc = tc.nc

    const_pool = ctx.enter_context(tc.tile_pool(name="const", bufs=1))
    ident = const_pool.tile([P, P], F32R)
    nc.scalar.dma_start(ident, ident_in)

    nat_pool = ctx.enter_context(tc.tile_pool(name="nat", bufs=2 * BPC))
    mt_pool = ctx.enter_context(tc.tile_pool(name="mt", bufs=2))
    tpsum = ctx.enter_context(tc.tile_pool(name="tpsum", bufs=2, space="PSUM"))
    mpsum = ctx.enter_context(tc.tile_pool(name="mpsum", bufs=6, space="PSUM"))
    outp = ctx.enter_context(tc.tile_pool(name="outp", bufs=6))

    def emit_loads(b, first):
        nats = []
        for name, src in (("m2", m2), ("m1", m1)):
            nat = nat_pool.tile([P, NT, D], F32R, tag="nat", name=f"nat_{name}_{b}")
            nchunk = 8 if (first and name == "m2") else 2
            ostep = NT // nchunk
            dma_eng = nc.sync if name == "m2" else nc.scalar
            for c in range(nchunk):
                dma_eng.dma_start(
                    nat[:, c * ostep : (c + 1) * ostep, :],
                    src[b].rearrange("(o p) d -> p o d", p=P)[
                        :, c * ostep : (c + 1) * ostep, :
                    ],
                )
            nats.append(nat)
        return nats

    def alloc_mts(b):
        return [
            mt_pool.tile([P, DC, R1], F32R, tag=name, name=f"{name}_{b}")
            for name in ("m2T", "m1T")
        ]

    def emit_transpose_unit(b, mi, nats, mts, o, start_eng):
        """Two PE transposes (dc=0,1) of one 128-row block, casts on
        alternating engines."""
        for dc in range(DC):
            ps = tpsum.tile([P, P], F32R, tag="tps", name=f"tps_{b}_{mi}_{o}_{dc}")
            nc.tensor.transpose(ps, nats[mi][:, o, dc * P : (dc + 1) * P], ident)
            dst = mts[mi][:, dc, o * P : (o + 1) * P]
            if (dc + start_eng) % 2 == 0:
                nc.vector.tensor_copy(dst, ps)
            else:
                nc.scalar.copy(dst, ps)

    def emit_mm_block(b, mts, it2, act_heavy=False):
        m2T, m1T = mts
        for half in range(2):
            it = it2 * 2 + half
            stage = outp.tile([P, R2], F32, tag="stage", name=f"stage_{b}_{it}")
            pss = [
                mpsum.tile([P, NJ_TILE], F32, tag="mps", name=f"mps_{b}_{it}_{jc}")
                for jc in range(NJ)
            ]
            for dc in range(DC):
                for jc in range(NJ):
                    nc.tensor.matmul(
                        pss[jc],
                        m1T[:, dc, it * P : (it + 1) * P],
                        m2T[:, dc, jc * NJ_TILE : (jc + 1) * NJ_TILE],
                        start=(dc == 0),
                        stop=(dc == DC - 1),
                    )
            for jc in range(NJ):
                dst = stage[:, jc * NJ_TILE : (jc + 1) * NJ_TILE]
                use_dve = (jc == 0) if act_heavy else (jc % 2 == 0)
                if use_dve:
                    nc.vector.tensor_copy(dst, pss[jc])
                else:
                    nc.scalar.copy(dst, pss[jc])
            nc.sync.dma_start(out[b, it * P : (it + 1) * P, :], stage)

    # all input loads issued up front (m2 on the sync ring, m1 on the
    # scalar ring) — no deps, so they pipeline ahead of the stores
    all_nats = [emit_loads(b, first=(b == 0)) for b in range(BPC)]
    all_mts = [alloc_mts(b) for b in range(BPC)]

    # batch-0 transposes upfront, alternating matrices so the PE can chew
    # m1 blocks (scalar-ring loads) while m2 chunks (sync ring) arrive
    eng = 0
    for mi in (0, 1):
        for o in range(NT):
            emit_transpose_unit(0, mi, all_nats[0], all_mts[0], o, eng)
            eng += 1

    for b in range(BPC):
        for it2 in range(NT // 2):
            emit_mm_block(b, all_mts[b], it2, act_heavy=(b + 1 < BPC and it2 < 4))
            # front-load next batch's transposes into the first 4 blocks,
            # while the DMA engines are still busy with input loads
            if b + 1 < BPC and it2 < 4:
                for o in range(4 * it2, 4 * it2 + 4):
                    for mi in (0, 1):
                        emit_transpose_unit(
                            b + 1, mi, all_nats[b + 1], all_mts[b + 1], o, eng
                        )
                        eng += 1


_NC_CACHE = None


def _build():
    global _NC_CACHE
    if _NC_CACHE is not None:
        return _NC_CACHE
    nc = bacc.Bacc(
        "TRN2", target_bir_lowering=False, debug=False, num_devices=NCORES
    )
    m1 = nc.dram_tensor("m1", [BPC, R1, D], F32R, kind="ExternalInput").ap()
    m2 = nc.dram_tensor("m2", [BPC, R2, D], F32R, kind="ExternalInput").ap()
    ident_in = nc.dram_tensor("ident", [P, P], F32R, kind="ExternalInput").ap()
    out = nc.dram_tensor("out", [BPC, R1, R2], F32, kind="ExternalOutput").ap()
    with tile.TileContext(nc) as tc:
        with ExitStack() as ctx:
            _build_tile_kernel(ctx, tc, m1, m2, ident_in, out)
    nc.compile()
    _NC_CACHE = nc
    return nc


def kernel(matrix_1: np.ndarray, matrix_2: np.ndarray, **run_kwargs) -> np.ndarray:
    m1 = np.ascontiguousarray(np.asarray(matrix_1, dtype=np.float32))
    m2 = np.ascontiguousarray(np.asarray(matrix_2, dtype=np.float32))
    assert m1.shape == (B, R1, D) and m2.shape == (B, R2, D)

    nc = _build()
    eye = np.eye(P, dtype=np.float32)
    in_maps = [
        {
            "m1": m1[i * BPC : (i + 1) * BPC],
            "m2": m2[i * BPC : (i + 1) * BPC],
            "ident": eye,
        }
        for i in range(NCORES)
    ]
    res = run_bass_kernel_spmd(
        nc, in_maps, core_ids=list(range(NCORES)), **run_kwargs
    )
    out = np.empty((B, R1, R2), dtype=np.float32)
    for i in range(NCORES):
        out[i * BPC : (i + 1) * BPC] = res.results[i]["out"]
    if run_kwargs:
        kernel.last_result = res
    return out



# revision 3
# speedup vs baseline: 1.1708x; 1.1708x over previous
"""Batched matrix-attention scores kernel for Trainium2 (8 NeuronCores).

Computes scores[b, i, j] = sum_d m1[b, i, d] * m2[b, j, d]
  (i.e. jnp.einsum('bid,bjd->bij', matrix_1, matrix_2))
with B=16, R1=R2=2048, D=256, fp32 in/out.

Sharding: data-parallel over batch - 2 batches per core on 8 cores.

Per-core HBM traffic is 8 MiB of loads + 32 MiB of stores; a single
HWDGE queue sustains ~420 GB/s, so the roofline is ~100 us. The
schedule is built to keep the DMA queues fed continuously:

  - b0 loads are split across both HWDGE rings (sync+scalar) in
    512 KiB quad-chunks; b1 loads follow on the scalar ring while
    stores own the sync ring.
  - Operands are PE-transposed (matmul transpose mode with identity)
    into D-on-partitions layout mT[d, dc, row]; transposes are packed
    4-to-a-PSUM-bank so one [128,512] copy evacuates a whole quad, and
    quads chase the load chunks so the first matmul block starts ~8us.
  - Each 128-row output tile takes 8 matmuls (2 d-chunks x 4 j-chunks
    of N=512, two 2-bank PSUM tiles); PSUM is evacuated split across
    DVE and ACT (one [128,1024] copy each) so store production stays
    above the DMA drain rate; output stored in 1 MiB blocks on the
    sync ring.
  - Remaining transpose quads (m1 of the current batch, both matrices
    of the next) are trickled between matmul blocks so the PE never
    lets the store queue drain.

Operands use dt.float32r (fp32 bits, full-rate single-pass PE matmul;
~2^-11 input mantissa truncation). Accumulation stays fp32 in PSUM.
"""

from contextlib import ExitStack

import numpy as np

import concourse.bass as bass
import concourse.mybir as mybir
import concourse.tile as tile
from concourse import bacc
from concourse.bass_utils import run_bass_kernel_spmd

F32 = mybir.dt.float32
F32R = mybir.dt.float32r

NCORES = 8
B, R1, R2, D = 16, 2048, 2048, 256
BPC = B // NCORES  # batches per core
P = 128
NJ_TILE = 512  # matmul free dim (one fp32 PSUM bank)
NJ = R2 // NJ_TILE  # j-chunks per row-block
NT = R1 // P  # 128-row tiles per batch
DC = D // P  # contraction chunks
NQ = NT // 4  # transpose quads (4 row-blocks) per (matrix, dc)
WARMUP_T = 12  # HAM warmup transposes


def _build_tile_kernel(ctx: ExitStack, tc: tile.TileContext, m1, m2, ident_in, out):
    nc = tc.nc

    const_pool = ctx.enter_context(tc.tile_pool(name="const", bufs=1))
    ident = const_pool.tile([P, P], F32R)
    nc.sync.dma_start(ident, ident_in)

    nat_pool = ctx.enter_context(tc.tile_pool(name="nat", bufs=1))
    mt_pool = ctx.enter_context(tc.tile_pool(name="mt", bufs=1))
    tpsum = ctx.enter_context(tc.tile_pool(name="tpsum", bufs=2, space="PSUM"))
    mpsum = ctx.enter_context(tc.tile_pool(name="mpsum", bufs=3, space="PSUM"))
    outp = ctx.enter_context(tc.tile_pool(name="outp", bufs=6))

    nat = {}
    mt = {}
    for b in range(BPC):
        for name in ("m2", "m1"):
            nat[(name, b)] = nat_pool.tile(
                [P, NT, D], F32R, tag=f"nat_{name}_{b}", name=f"nat_{name}_{b}"
            )
            mt[(name, b)] = mt_pool.tile(
                [P, DC, R1], F32R, tag=f"mt_{name}_{b}", name=f"mt_{name}_{b}"
            )

    def load_chunk(eng, name, b, q):
        """One 512 KiB quad-chunk (4 row-blocks) of a matrix into nat."""
        src = m2 if name == "m2" else m1
        eng.dma_start(
            nat[(name, b)][:, q * 4 : (q + 1) * 4, :],
            src[b].rearrange("(o p) d -> p o d", p=P)[:, q * 4 : (q + 1) * 4, :],
        )

    t_toggle = [0]

    def t_quad(name, b, q, dc):
        """Transpose 4 row-blocks (one d-chunk) into one PSUM bank, then
        evacuate with a single [128,512] copy on alternating engines."""
        tp = tpsum.tile([P, NJ_TILE], F32R, tag="tp", name=f"tp_{name}_{b}_{q}_{dc}")
        for k in range(4):
            o = q * 4 + k
            nc.tensor.transpose(
                tp[:, k * P : (k + 1) * P],
                nat[(name, b)][:, o, dc * P : (dc + 1) * P],
                ident,
            )
        dst = mt[(name, b)][:, dc, q * NJ_TILE : (q + 1) * NJ_TILE]
        if t_toggle[0] % 2 == 0:
            nc.vector.tensor_copy(dst, tp)
        else:
            nc.scalar.copy(dst, tp)
        t_toggle[0] += 1

    def t_pair(name, b, q):
        for dc in range(DC):
            t_quad(name, b, q, dc)

    def mm_block(b, it):
        """One 128-row output tile: 8 matmuls into two 2-bank PSUM tiles,
        evacuated in parallel halves on DVE and ACT, stored on sync."""
        m2T, m1T = mt[("m2", b)], mt[("m1", b)]
        stage = outp.tile([P, R2], F32, tag="stage", name=f"stage_{b}_{it}")
        for half in range(2):
            ps = mpsum.tile(
                [P, 2 * NJ_TILE], F32, tag="mm", name=f"mps_{b}_{it}_{half}"
            )
            for jl in range(2):
                jc = half * 2 + jl
                for dc in range(DC):
                    nc.tensor.matmul(
                        ps[:, jl * NJ_TILE : (jl + 1) * NJ_TILE],
                        m1T[:, dc, it * P : (it + 1) * P],
                        m2T[:, dc, jc * NJ_TILE : (jc + 1) * NJ_TILE],
                        start=(dc == 0),
                        stop=(dc == DC - 1),
                    )
            dst = stage[:, half * 2 * NJ_TILE : (half + 1) * 2 * NJ_TILE]
            if half == 0:
                nc.vector.tensor_copy(dst, ps)
            else:
                nc.scalar.copy(dst, ps)
        nc.sync.dma_start(out[b, it * P : (it + 1) * P, :], stage)

    # ---- loads ----
    # b0 is ramp-critical: split across sync+scalar rings. m2b1 follows on
    # sync (dispatches are dep-free, so they issue before the stores).
    # m1b1 rides the otherwise-idle gpsimd (SWDGE) queue, held back by a
    # tiny dependency on the last m1b0 chunk so it doesn't steal ramp
    # bandwidth from b0.
    for name, b, q in (
        ("m2", 0, 0),
        ("m2", 0, 2),
        ("m1", 0, 1),
        ("m1", 0, 3),
        ("m2", 1, 0),
        ("m2", 1, 1),
        ("m2", 1, 2),
        ("m2", 1, 3),
    ):
        load_chunk(nc.sync, name, b, q)
    for name, b, q in (("m2", 0, 1), ("m2", 0, 3), ("m1", 0, 0), ("m1", 0, 2)):
        load_chunk(nc.scalar, name, b, q)
    gp_scratch = const_pool.tile([P, 4], F32R, tag="gps", name="gp_scratch")
    nc.gpsimd.tensor_copy(gp_scratch, nat[("m1", 0)][:, NT - 1, 0:4])
    for q in range(NQ):
        load_chunk(nc.gpsimd, "m1", 1, q)

    # ---- HAM warmup: dummy transposes on the identity ----
    for w in range(WARMUP_T):
        wtp = tpsum.tile([P, NJ_TILE], F32R, tag="tp", name=f"warm_{w}")
        nc.tensor.transpose(wtp[:, 0:P], ident, ident)

    # ---- b0 m2 transposes chase the load chunks ----
    for q in range(NQ):
        t_pair("m2", 0, q)
    # first m1 quad, then matmul blocks with trickled transposes
    t_pair("m1", 0, 0)

    # schedule: after b0 block k, emit these transpose pairs/quads
    after_b0 = {
        0: [("m1", 0, 1)],
        2: [("m1", 0, 2)],
        4: [("m1", 0, 3)],
    }
    # b1 m2 quads (8 singles) after b0 blocks 6..13
    b1_m2 = [("m2", 1, q, dc) for q in range(NQ) for dc in range(DC)]
    after_b1 = {
        14: [("m1", 1, 0)],
        15: [("m1", 1, 1)],
    }
    after_b1_blocks = {
        1: [("m1", 1, 2)],
        3: [("m1", 1, 3)],
    }

    for it in range(NT):
        mm_block(0, it)
        for name, b, q in after_b0.get(it, []):
            t_pair(name, b, q)
        if 6 <= it <= 13:
            name, b, q, dc = b1_m2[it - 6]
            t_quad(name, b, q, dc)
        for name, b, q in after_b1.get(it, []):
            t_pair(name, b, q)

    for it in range(NT):
        mm_block(1, it)
        for name, b, q in after_b1_blocks.get(it, []):
            t_pair(name, b, q)


_NC_CACHE = None


def _build():
    global _NC_CACHE
    if _NC_CACHE is not None:
        return _NC_CACHE
    nc = bacc.Bacc(
        "TRN2", target_bir_lowering=False, debug=False, num_devices=NCORES
    )
    m1 = nc.dram_tensor("m1", [BPC, R1, D], F32R, kind="ExternalInput").ap()
    m2 = nc.dram_tensor("m2", [BPC, R2, D], F32R, kind="ExternalInput").ap()
    ident_in = nc.dram_tensor("ident", [P, P], F32R, kind="ExternalInput").ap()
    out = nc.dram_tensor("out", [BPC, R1, R2], F32, kind="ExternalOutput").ap()
    with tile.TileContext(nc) as tc:
        with ExitStack() as ctx:
            _build_tile_kernel(ctx, tc, m1, m2, ident_in, out)
    nc.compile()
    _NC_CACHE = nc
    return nc


def kernel(matrix_1: np.ndarray, matrix_2: np.ndarray, **run_kwargs) -> np.ndarray:
    m1 = np.ascontiguousarray(np.asarray(matrix_1, dtype=np.float32))
    m2 = np.ascontiguousarray(np.asarray(matrix_2, dtype=np.float32))
    assert m1.shape == (B, R1, D) and m2.shape == (B, R2, D)

    nc = _build()
    eye = np.eye(P, dtype=np.float32)
    in_maps = [
        {
            "m1": m1[i * BPC : (i + 1) * BPC],
            "m2": m2[i * BPC : (i + 1) * BPC],
            "ident": eye,
        }
        for i in range(NCORES)
    ]
    res = run_bass_kernel_spmd(
        nc, in_maps, core_ids=list(range(NCORES)), **run_kwargs
    )
    out = np.empty((B, R1, R2), dtype=np.float32)
    for i in range(NCORES):
        out[i * BPC : (i + 1) * BPC] = res.results[i]["out"]
    if run_kwargs:
        kernel.last_result = res
    return out
